# revision 7
# baseline (speedup 1.0000x reference)
"""Trainium2 Bass kernel for nn_MultiHeadAttention_54460185313542.

Multi-head attention: B=4, S=2048, D=1024, H=16 heads (HD=64), fp32 I/O.

Sharding (8 cores): core c handles batch b = c//2 and heads
[hb*8, hb*8+8) where hb = c%2 (data parallel over batch x tensor
parallel over head halves). Each core computes a partial fc output
y_c = attn_slice @ Wf.T_slice; the host sums the two partials per batch
and adds the fc bias.

Per-core device algorithm (transposed-activation layout):
  - qT = WqT_slice contracted with xT -> [512, S] (head dim on partitions)
  - kT likewise; v natural [S, 512] stored per head with a ones column
    PREPENDED (softmax-sum trick: PSUM row 0 of the p@v output collects
    the softmax denominator, rows 1..64 the unnormalized context).
  - per head h, per qq-block: scoresT[kk,qq] in PSUM (K=64 matmuls), exp
    on ScalarE (scale=1/8 folded in; no max subtraction -- scores are
    ~N(0,1) so exp cannot overflow; identical result after normalization),
    mask multiply on VectorE (mask pre-transposed/cast to fp16 on host),
    p@v via K=128 matmuls into the augmented PSUM tile.
  - normalization: sums row 0 -> fast reciprocal -> partition_broadcast
    to rows 1..64 -> fused into the PSUM->SBUF evacuation multiply; the
    [64, QB] staging tile is DMA-packed into [128, S] head-pair tiles.
  - fc: y[qq, o] = sum_pairs outT_pair.T-contract WfT.

All matmul operands fp16 (full PE rate), all accumulation fp32 in PSUM.
"""

import os
import sys
from contextlib import ExitStack

for _p in ("/root/.axon_site/_ro/trn_rl_repo", "/opt/trn_rl_repo"):
    if os.path.isdir(_p) and _p not in sys.path:
        sys.path.append(_p)

import numpy as np

import concourse.bass as bass  # noqa: E402,F401
import concourse.tile as tile  # noqa: E402
import concourse.mybir as mybir  # noqa: E402
from concourse import bacc  # noqa: E402
from concourse.bass_utils import run_bass_kernel_spmd  # noqa: E402

# Problem constants
B, S_FULL, D, H, HD = 4, 2048, 1024, 16, 64
NCORES = 8
HPC = H // 2          # 8 heads per core
DPC = HPC * HD        # 512: slice of D handled per core
P = 128

F16 = mybir.dt.float16
F32 = mybir.dt.float32
ALU = mybir.AluOpType
AFT = mybir.ActivationFunctionType


def build_program(S=S_FULL, QB=1024):
    """Build + bacc-compile the single-core Bass program (same NEFF runs
    SPMD on all 8 cores with different input data)."""
    QB = min(QB, S)
    nc = bacc.Bacc("TRN2", target_bir_lowering=False, debug=False,
                   num_devices=NCORES)

    xT = nc.dram_tensor("xT", [D, S], F16, kind="ExternalInput")
    maskT = nc.dram_tensor("maskT", [S, S], F16, kind="ExternalInput")
    wqT = nc.dram_tensor("wqT", [D, DPC], F16, kind="ExternalInput")
    wkT = nc.dram_tensor("wkT", [D, DPC], F16, kind="ExternalInput")
    wvT = nc.dram_tensor("wvT", [D, DPC], F16, kind="ExternalInput")
    wfT = nc.dram_tensor("wfT", [DPC, D], F16, kind="ExternalInput")
    bq = nc.dram_tensor("bq", [DPC], F32, kind="ExternalInput")
    bk = nc.dram_tensor("bk", [DPC], F32, kind="ExternalInput")
    bvr = nc.dram_tensor("bvr", [P, DPC], F32, kind="ExternalInput")
    y = nc.dram_tensor("y", [S, D], F32, kind="ExternalOutput")

    tensors = dict(xT=xT, maskT=maskT, wqT=wqT, wkT=wkT, wvT=wvT, wfT=wfT,
                   bq=bq, bk=bk, bvr=bvr, y=y)
    with tile.TileContext(nc, pool_alloc_mode="queue") as tc:
        _body(tc, tensors, S, QB)

    nc.compile()
    return nc


def _body(tc, t, S, QB):
    nc = tc.nc
    xT, maskT = t["xT"], t["maskT"]
    wqT, wkT, wvT, wfT = t["wqT"], t["wkT"], t["wvT"], t["wfT"]
    bq, bk, bvr, y = t["bq"], t["bk"], t["bvr"], t["y"]

    NKK = S // P          # kk tiles (key dim)
    NQB = S // QB         # qq blocks per row
    NJ = D // P           # contraction tiles over D
    NM = DPC // P         # head-pair tiles (4)
    NH = max(QB // 512, 1)  # 512-wide matmul slices per qq block
    MW = QB // NH           # matmul free width (<=512)
    NS = S // MW
    NKA = NKK // 2          # mask kk-tiles in pool A (exact-fit pool swap)

    with ExitStack() as ctx:
        mpoolA = ctx.enter_context(tc.tile_pool(name="mpoolA", bufs=1))
        constp = ctx.enter_context(tc.tile_pool(name="constp", bufs=1))
        wpool = ctx.enter_context(tc.tile_pool(name="wpool", bufs=1))
        qkvp = ctx.enter_context(tc.tile_pool(name="qkvp", bufs=1))
        outp = ctx.enter_context(tc.tile_pool(name="outp", bufs=1))
        work = ctx.enter_context(tc.tile_pool(name="work", bufs=3))
        repp = ctx.enter_context(tc.tile_pool(name="repp", bufs=2))
        srp = ctx.enter_context(tc.tile_pool(name="srp", bufs=2))
        yp = ctx.enter_context(tc.tile_pool(name="yp", bufs=2))
        psp = ctx.enter_context(tc.tile_pool(name="psp", bufs=2, space="PSUM"))
        paug = ctx.enter_context(tc.tile_pool(name="paug", bufs=1, space="PSUM"))
        pproj = ctx.enter_context(tc.tile_pool(name="pproj", bufs=2, space="PSUM"))
        # xpool allocated last (stack-ordered release): freed after the
        # projections; its ring zone is reused by the mask B pool.
        xpool = tc.alloc_tile_pool(name="xpool", bufs=1)

        # ---- weights / constants ----
        wq_sb = wpool.tile([P, NJ, DPC], F16)
        wk_sb = wpool.tile([P, NJ, DPC], F16)
        wv_sb = wpool.tile([P, NJ, DPC], F16)
        wf_sb = wpool.tile([P, NM, D], F16)
        wq_r = wqT.ap().rearrange("(o p) m -> o p m", p=P)
        wk_r = wkT.ap().rearrange("(o p) m -> o p m", p=P)
        wv_r = wvT.ap().rearrange("(o p) m -> o p m", p=P)
        wf_r = wfT.ap().rearrange("(o p) m -> o p m", p=P)
        for o in range(NJ):
            nc.sync.dma_start(wq_sb[:, o], wq_r[o])
            nc.sync.dma_start(wk_sb[:, o], wk_r[o])
            nc.sync.dma_start(wv_sb[:, o], wv_r[o])
        for o in range(NM):
            nc.sync.dma_start(wf_sb[:, o], wf_r[o])
        bq_sb = constp.tile([P, NM], F32)
        bk_sb = constp.tile([P, NM], F32)
        bv_sb = constp.tile([P, DPC], F32)
        nc.sync.dma_start(bq_sb, bq.ap().rearrange("(m p) -> p m", p=P))
        nc.sync.dma_start(bk_sb, bk.ap().rearrange("(m p) -> p m", p=P))
        nc.sync.dma_start(bv_sb, bvr.ap())

        # first half of the mask (loads overlap the projection phase)
        mr = maskT.ap().rearrange("(o p) s -> o p s", p=P)
        mA_sb = mpoolA.tile([P, NKA, S], F16)
        for o in range(NKA):
            nc.sync.dma_start(mA_sb[:, o], mr[o])

        qT_sb = qkvp.tile([P, NM, S], F16)
        kT_sb = qkvp.tile([P, NM, S], F16)
        v_sb = qkvp.tile([P, NKK, HPC, HD + 1], F16)
        nc.vector.memset(v_sb[:, :, :, HD:HD + 1], 1.0)

        # ---- projections ----
        # qk-proj for pair m and v-proj emitted as "groups" (one PSUM tile +
        # NJ matmuls + one evacuation); pair-0 qk runs up front, everything
        # else is interleaved into attention PE slack.
        xT_sb = xpool.tile([P, NJ, S], F16)
        xr = xT.ap().rearrange("(o p) s -> o p s", p=P)
        for o in range(NJ):
            nc.sync.dma_start(xT_sb[:, o], xr[o])

        def qk_group(m, mat, n):
            wsb, bsb, dst = ((wq_sb, bq_sb, qT_sb), (wk_sb, bk_sb, kT_sb))[mat]
            pt = pproj.tile([P, 512], F32, tag="proj", name="pt_qk")
            for k in range(NJ):
                nc.tensor.matmul(pt, wsb[:, k, m * P:(m + 1) * P],
                                 xT_sb[:, k, n * 512:(n + 1) * 512],
                                 start=(k == 0), stop=(k == NJ - 1))
            nc.vector.tensor_scalar(out=dst[:, m, n * 512:(n + 1) * 512],
                                    in0=pt, scalar1=bsb[:, m:m + 1],
                                    scalar2=None, op0=ALU.add)

        def v_group(m2):
            pt = pproj.tile([P, 512], F32, tag="proj", name="pt_v")
            for k in range(NJ):
                nc.tensor.matmul(pt, xT_sb[:, k, m2 * P:(m2 + 1) * P],
                                 wv_sb[:, k, :],
                                 start=(k == 0), stop=(k == NJ - 1))
            nc.vector.scalar_tensor_tensor(
                out=v_sb[:, m2, :, :HD], in0=pt, scalar=1.0,
                in1=bv_sb.rearrange("p (h d) -> p h d", h=HPC),
                op0=ALU.mult, op1=ALU.add)

        NB = S // 512           # 512-wide qq blocks (and n-blocks)
        for n in range(NB):
            qk_group(0, 0, n)
            qk_group(0, 1, n)

        # mask pool B after xpool release would deadlock proj interleaving,
        # so xpool is released only after pair 0 (which consumes xT for the
        # interleaved v/qk groups); mpoolB opens then.  Pair 0 uses mask
        # tiles from pool A only for kk < NKA; kk >= NKA tiles are loaded
        # into pool B before pairs >= 1... but pair 0 also needs them.
        # Simplest correct scheme: pair 0 consumes all interleaved proj
        # groups; release xpool; open mpoolB; then run ALL pairs' attention
        # including pair 0.  Instead we interleave proj into pair 0's
        # attention and give mpoolB tiles only to kk >= NKA: those arrive
        # while pair-0 kk < NKA iterations run.

        # ---- attention ----
        pairs_sb = outp.tile([P, NM, S], F16)
        state = {"mB": None}

        def mtile(kk):
            return mA_sb[:, kk] if kk < NKA else state["mB"][:, kk - NKA]

        def attn_pair(pr, hooks=None):
            hA, hB = 2 * pr, 2 * pr + 1
            for qb in range(NB):
                q0 = qb * 512
                augA = paug.tile([P, 512], F32, tag="augA", name="augA")
                augB = paug.tile([P, 512], F32, tag="augB", name="augB")
                prev = None
                for kk in range(NKK):
                    if hooks:
                        for hk in hooks.get((qb, kk), ()):
                            hk()
                    ps_t = psp.tile([P, 1024], F32, tag="ps", name="ps_s")
                    nc.tensor.matmul(ps_t[:, 0:512],
                                     kT_sb[0:64, pr, kk * P:(kk + 1) * P],
                                     qT_sb[0:64, pr, q0:q0 + 512],
                                     start=True, stop=True)
                    nc.tensor.matmul(ps_t[:, 512:1024],
                                     kT_sb[64:128, pr, kk * P:(kk + 1) * P],
                                     qT_sb[64:128, pr, q0:q0 + 512],
                                     start=True, stop=True)
                    if prev is not None:
                        p_prev, kkp = prev
                        nc.tensor.matmul(augA[:HD + 1, :],
                                         v_sb[:, kkp, hA, :], p_prev[:, 0:512],
                                         start=(kkp == 0), stop=(kkp == NKK - 1))
                        nc.tensor.matmul(augB[:HD + 1, :],
                                         v_sb[:, kkp, hB, :], p_prev[:, 512:1024],
                                         start=(kkp == 0), stop=(kkp == NKK - 1))
                    p_t = work.tile([P, 1024], F16, tag="p", name="p_t")
                    nc.scalar.activation(p_t, ps_t, AFT.Exp, scale=0.125)
                    m_ap = mtile(kk)[:, q0:q0 + 512]
                    nc.vector.tensor_tensor(p_t[:, 0:512], p_t[:, 0:512],
                                            m_ap, ALU.mult)
                    nc.vector.tensor_tensor(p_t[:, 512:1024], p_t[:, 512:1024],
                                            m_ap, ALU.mult)
                    prev = (p_t, kk)
                p_prev, kkp = prev
                nc.tensor.matmul(augA[:HD + 1, :],
                                 v_sb[:, kkp, hA, :], p_prev[:, 0:512],
                                 start=(kkp == 0), stop=(kkp == NKK - 1))
                nc.tensor.matmul(augB[:HD + 1, :],
                                 v_sb[:, kkp, hB, :], p_prev[:, 512:1024],
                                 start=(kkp == 0), stop=(kkp == NKK - 1))

                for aug, pbase in ((augA, 0), (augB, 64)):
                    srow_t = srp.tile([1, 512], F32, tag="sr", name="srow_t")
                    nc.vector.tensor_copy(out=srow_t, in_=aug[HD:HD + 1, :])
                    nc.vector.reciprocal_approx_fast(out=srow_t, in_=srow_t)
                    rep_t = repp.tile([64, 512], F32, tag="rep", name="rep_t")
                    nc.gpsimd.partition_broadcast(rep_t, srow_t)
                    nc.vector.tensor_tensor(
                        pairs_sb[pbase:pbase + 64, pr, q0:q0 + 512],
                        aug[:HD, :], rep_t, ALU.mult)

        # pair 0 with v-proj groups interleaved in its first qq block and
        # remaining qk-proj groups spread over the later blocks.
        hooks0 = {}
        for kk in range(NKK):
            hooks0.setdefault((0, kk % NKK), []).append(
                (lambda m2: lambda: v_group(m2))(kk))
        gi = 0
        for m in (1, 2, 3):
            for mat in (0, 1):
                for n in range(NB):
                    qb = 1 + gi // NKK
                    kk = gi % NKK
                    if qb >= NB:
                        qb, kk = NB - 1, NKK - 1
                    hooks0.setdefault((qb, kk), []).append(
                        (lambda a, b, c: lambda: qk_group(a, b, c))(m, mat, n))
                    gi += 2
        attn_pair(0, hooks0)

        xpool.release()
        mpoolB = ctx.enter_context(tc.tile_pool(name="mpoolB", bufs=1))
        mB_sb = mpoolB.tile([P, NKK - NKA, S], F16)
        state["mB"] = mB_sb
        for o in range(NKA, NKK):
            nc.sync.dma_start(mB_sb[:, o - NKA], mr[o])

        for pr in (1, 2, 3):
            attn_pair(pr)

        # ---- fc ----
        for qt in range(S // P):
            for ot in range(D // 512):
                pt = pproj.tile([P, 512], F32, tag="proj", name="pt_fc")
                for pr2 in range(NM):
                    nc.tensor.matmul(pt, pairs_sb[:, pr2, qt * P:(qt + 1) * P],
                                     wf_sb[:, pr2, ot * 512:(ot + 1) * 512],
                                     start=(pr2 == 0), stop=(pr2 == NM - 1))
                y_t = yp.tile([P, 512], F32, tag="y", name="y_t")
                nc.scalar.copy(y_t, pt)
                nc.sync.dma_start(
                    y.ap()[qt * P:(qt + 1) * P, ot * 512:(ot + 1) * 512], y_t)


_prog_cache = {}


def _get_program(S=S_FULL, QB=1024):
    key = (S, QB)
    if key not in _prog_cache:
        _prog_cache[key] = build_program(S, QB)
    return _prog_cache[key]


def make_in_maps(x, mask, Wq, bq, Wk, bk, Wv, bv, Wf, bf):
    """Host-side sharding + layout prep (fp16 casts / transposes)."""
    in_maps = []
    WqT = np.ascontiguousarray(np.asarray(Wq).T.astype(np.float16))
    WkT = np.ascontiguousarray(np.asarray(Wk).T.astype(np.float16))
    WvT = np.ascontiguousarray(np.asarray(Wv).T.astype(np.float16))
    WfT = np.asarray(Wf).T.astype(np.float16)
    bq = np.asarray(bq, dtype=np.float32)
    bk = np.asarray(bk, dtype=np.float32)
    bv = np.asarray(bv, dtype=np.float32)
    x = np.asarray(x)
    mask = np.asarray(mask)
    for c in range(NCORES):
        b, hb = c // 2, c % 2
        hs = slice(hb * DPC, (hb + 1) * DPC)
        in_maps.append({
            "xT": np.ascontiguousarray(x[b].T.astype(np.float16)),
            "maskT": np.ascontiguousarray(mask[b, 0].T.astype(np.float16)),
            "wqT": np.ascontiguousarray(WqT[:, hs]),
            "wkT": np.ascontiguousarray(WkT[:, hs]),
            "wvT": np.ascontiguousarray(WvT[:, hs]),
            "wfT": np.ascontiguousarray(WfT[hs, :]),
            "bq": np.ascontiguousarray(bq[hs]),
            "bk": np.ascontiguousarray(bk[hs]),
            "bvr": np.ascontiguousarray(np.tile(bv[hs][None, :], (P, 1))),
        })
    return in_maps


def kernel(x, mask, Wq, bq, Wk, bk, Wv, bv, Wf, bf, _trace=False):
    nc = _get_program()
    in_maps = make_in_maps(x, mask, Wq, bq, Wk, bk, Wv, bv, Wf, bf)
    res = run_bass_kernel_spmd(nc, in_maps, core_ids=list(range(NCORES)),
                               trace=_trace)
    out = np.empty((B, S_FULL, D), np.float32)
    bf32 = np.asarray(bf, dtype=np.float32)
    for b in range(B):
        out[b] = res.results[2 * b]["y"] + res.results[2 * b + 1]["y"] + bf32[None, :]
    if _trace:
        kernel._last_results = res
    return out


# revision 22
# speedup vs baseline: 10377.3163x; 10377.3163x over previous
"""Trainium2 Bass kernel for nn_MultiHeadAttention_54460185313542.

Multi-head attention: B=4, S=2048, D=1024, H=16 heads (HD=64), fp32 I/O.

Sharding (8 cores): core c handles batch b = c//2 and heads
[hb*8, hb*8+8) where hb = c%2 (data parallel over batch x tensor
parallel over head halves). Each core computes a partial fc output
y_c = attn_slice @ Wf.T_slice; the host sums the two partials per batch
and adds the fc bias.

Per-core device algorithm (transposed-activation layout):
  - qT = WqT_slice contracted with xT -> [512, S] (head dim on partitions,
    head pairs packed 64+64 into 128-partition tiles); kT likewise; v
    natural [S, 512] per head with a ones column appended (softmax-sum
    trick: PSUM row 64 of the p@v output collects the denominator).
  - attention runs per head-PAIR: for each 512-wide qq block and each
    128-wide kk tile, the two heads' scoresT land in the two halves of one
    [128, 1024] PSUM tile via two row-tiled K=64 matmuls (row groups 0-1 /
    2-3, concurrent in the PE array).  One Exp on ScalarE covers both
    heads (scale=1/8 folded in; no max subtraction -- scores are ~N(0,1)
    so exp cannot overflow; identical after normalization).  Mask multiply
    on VectorE (mask pre-transposed/cast to fp16 on host).  p@v via K=128
    matmuls into per-head augmented PSUM tiles (lagged one kk so the PE
    never waits on exp/mask).
  - normalization: sums row -> fast reciprocal -> partition_broadcast ->
    fused into the PSUM->SBUF evacuation multiply.
  - projections are emitted as groups (PSUM tile + 8 matmuls + evac) and
    interleaved into the attention loop's PE slack (v during pair 0,
    qk(m) during pair m-1), using 2 dedicated PSUM banks.
  - fc: y[qq, o] = sum_pairs outT_pair.T-contract WfT (Wf streamed).

All matmul operands fp16 (full PE rate), all accumulation fp32 in PSUM.
"""

import os
import sys
from contextlib import ExitStack

for _p in ("/root/.axon_site/_ro/trn_rl_repo", "/opt/trn_rl_repo"):
    if os.path.isdir(_p) and _p not in sys.path:
        sys.path.append(_p)

import numpy as np

import concourse.bass as bass  # noqa: E402,F401
import concourse.tile as tile  # noqa: E402
import concourse.mybir as mybir  # noqa: E402
from concourse import bacc  # noqa: E402
from concourse.bass_utils import run_bass_kernel_spmd  # noqa: E402

# Problem constants
B, S_FULL, D, H, HD = 4, 2048, 1024, 16, 64
NCORES = 8
HPC = H // 2          # 8 heads per core
DPC = HPC * HD        # 512: slice of D handled per core
P = 128

F16 = mybir.dt.float16
F32 = mybir.dt.float32
ALU = mybir.AluOpType
AFT = mybir.ActivationFunctionType


def build_program(S=S_FULL):
    """Build + bacc-compile the single-core Bass program (same NEFF runs
    SPMD on all 8 cores with different input data)."""
    nc = bacc.Bacc("TRN2", target_bir_lowering=False, debug=False,
                   num_devices=NCORES)

    xT = nc.dram_tensor("xT", [D, S], F16, kind="ExternalInput")
    maskT = nc.dram_tensor("maskT", [S, S], F16, kind="ExternalInput")
    wqT = nc.dram_tensor("wqT", [D, DPC], F16, kind="ExternalInput")
    wkT = nc.dram_tensor("wkT", [D, DPC], F16, kind="ExternalInput")
    wvT = nc.dram_tensor("wvT", [D, DPC], F16, kind="ExternalInput")
    wfT = nc.dram_tensor("wfT", [DPC, D], F16, kind="ExternalInput")
    bq = nc.dram_tensor("bq", [DPC], F32, kind="ExternalInput")
    bk = nc.dram_tensor("bk", [DPC], F32, kind="ExternalInput")
    bvr = nc.dram_tensor("bvr", [P, DPC], F16, kind="ExternalInput")
    y = nc.dram_tensor("y", [S, D], F32, kind="ExternalOutput")

    tensors = dict(xT=xT, maskT=maskT, wqT=wqT, wkT=wkT, wvT=wvT, wfT=wfT,
                   bq=bq, bk=bk, bvr=bvr, y=y)
    with tile.TileContext(nc, pool_alloc_mode="queue") as tc:
        _body(tc, tensors, S)

    nc.compile()
    return nc


def _body(tc, t, S):
    nc = tc.nc
    xT, maskT = t["xT"], t["maskT"]
    wqT, wkT, wvT, wfT = t["wqT"], t["wkT"], t["wvT"], t["wfT"]
    bq, bk, bvr, y = t["bq"], t["bk"], t["bvr"], t["y"]

    NKK = S // P          # kk tiles (key dim)
    NJ = D // P           # contraction tiles over D
    NM = DPC // P         # head-pair tiles (4)
    NB = S // 512         # 512-wide qq / token blocks

    with ExitStack() as ctx:
        mpool = ctx.enter_context(tc.tile_pool(name="mpool", bufs=1))
        xpool = ctx.enter_context(tc.tile_pool(name="xpool", bufs=1))
        constp = ctx.enter_context(tc.tile_pool(name="constp", bufs=1))
        wpool = ctx.enter_context(tc.tile_pool(name="wpool", bufs=1))
        qkvp = ctx.enter_context(tc.tile_pool(name="qkvp", bufs=1))
        qkp = ctx.enter_context(tc.tile_pool(name="qkp", bufs=2))
        outp = ctx.enter_context(tc.tile_pool(name="outp", bufs=1))
        work = ctx.enter_context(tc.tile_pool(name="work", bufs=2))
        repp = ctx.enter_context(tc.tile_pool(name="repp", bufs=1))
        srp = ctx.enter_context(tc.tile_pool(name="srp", bufs=1))
        yp = ctx.enter_context(tc.tile_pool(name="yp", bufs=2))
        psp = ctx.enter_context(tc.tile_pool(name="psp", bufs=2, space="PSUM"))
        paug = ctx.enter_context(tc.tile_pool(name="paug", bufs=1, space="PSUM"))
        pproj = ctx.enter_context(tc.tile_pool(name="pproj", bufs=2, space="PSUM"))

        # ---- input loads (xT first: the lead projection groups need it) ----
        xT_sb = xpool.tile([P, NJ, S], F16)
        xr = xT.ap().rearrange("(o p) s -> o p s", p=P)
        H4 = S // 4
        for o in range(NJ):
            for q_ in range(4):
                nc.sync.dma_start(xT_sb[:, o, q_ * H4:(q_ + 1) * H4],
                                  xr[o][:, q_ * H4:(q_ + 1) * H4])
        wq_sb = wpool.tile([P, NJ, DPC], F16)
        wk_sb = wpool.tile([P, NJ, DPC], F16)
        wv_sb = wpool.tile([P, NJ, DPC], F16)
        wf_sb = wpool.tile([P, NM, D], F16)
        wq_r = wqT.ap().rearrange("(o p) m -> o p m", p=P)
        wk_r = wkT.ap().rearrange("(o p) m -> o p m", p=P)
        wv_r = wvT.ap().rearrange("(o p) m -> o p m", p=P)
        wf_r = wfT.ap().rearrange("(o p) m -> o p m", p=P)
        for o in range(NJ):
            nc.sync.dma_start(wk_sb[:, o], wk_r[o])
            nc.sync.dma_start(wq_sb[:, o], wq_r[o])
            nc.sync.dma_start(wv_sb[:, o], wv_r[o])
        bq_sb = constp.tile([P, NM], F32)
        bk_sb = constp.tile([P, NM], F32)
        bv_sb = constp.tile([P, DPC], F16)
        nc.sync.dma_start(bq_sb, bq.ap().rearrange("(m p) -> p m", p=P))
        nc.sync.dma_start(bk_sb, bk.ap().rearrange("(m p) -> p m", p=P))
        nc.sync.dma_start(bv_sb, bvr.ap())
        mT_sb = mpool.tile([P, NKK, S], F16)
        mr = maskT.ap().rearrange("(o p) s -> o p s", p=P)
        for o in range(NKK):
            nc.sync.dma_start(mT_sb[:, o], mr[o])
        for o in range(NM):
            nc.sync.dma_start(wf_sb[:, o], wf_r[o])

        v_sb = qkvp.tile([P, NKK, HPC, HD + 1], F16)
        nc.vector.memset(v_sb[:, :, :, HD:HD + 1], 1.0)

        # ---- projection groups ----
        # qT/kT live per-pair in a 2-deep rotation (current + next pair)
        qk_tiles = {}

        def qk_pair_tiles(m):
            if m not in qk_tiles:
                q_t = qkp.tile([P, S], F16, tag="qt", name=f"q_t{m}")
                k_t = qkp.tile([P, S], F16, tag="kt", name=f"k_t{m}")
                qk_tiles[m] = (q_t, k_t)
            return qk_tiles[m]

        def qk_group(m, mat, n):
            wsb, bsb = ((wq_sb, bq_sb), (wk_sb, bk_sb))[mat]
            dst = qk_pair_tiles(m)[mat]
            pt = pproj.tile([P, 512], F32, tag="proj", name="pt_qk")
            for k in range(NJ):
                nc.tensor.matmul(pt, wsb[:, k, m * P:(m + 1) * P],
                                 xT_sb[:, k, n * 512:(n + 1) * 512],
                                 start=(k == 0), stop=(k == NJ - 1))
            nc.vector.tensor_scalar(out=dst[:, n * 512:(n + 1) * 512],
                                    in0=pt, scalar1=bsb[:, m:m + 1],
                                    scalar2=None, op0=ALU.add)

        def v_group(m2):
            pt = pproj.tile([P, 512], F32, tag="proj", name="pt_v")
            for k in range(NJ):
                nc.tensor.matmul(pt, xT_sb[:, k, m2 * P:(m2 + 1) * P],
                                 wv_sb[:, k, :],
                                 start=(k == 0), stop=(k == NJ - 1))
            nc.vector.scalar_tensor_tensor(
                out=v_sb[:, m2, :, :HD], in0=pt, scalar=1.0,
                in1=bv_sb.rearrange("p (h d) -> p h d", h=HPC),
                op0=ALU.mult, op1=ALU.add)

        def fc_group(qt, ot):
            pt = pproj.tile([P, 512], F32, tag="proj", name="pt_fc")
            for pr2 in range(NM):
                nc.tensor.matmul(pt, pairs_sb[:, pr2, qt * P:(qt + 1) * P],
                                 wf_sb[:, pr2, ot * 512:(ot + 1) * 512],
                                 start=(pr2 == 0), stop=(pr2 == NM - 1))
            y_t = yp.tile([P, 512], F32, tag="y", name="y_t")
            nc.vector.tensor_copy(out=y_t, in_=pt)
            nc.sync.dma_start(
                y.ap()[qt * P:(qt + 1) * P, ot * 512:(ot + 1) * 512], y_t)

        # lead-in: just enough projection for the first scores
        qk_group(0, 1, 0)
        qk_group(0, 0, 0)

        # ---- attention: one software-pipelined stream over all steps ----
        pairs_sb = outp.tile([P, NM, S], F16)

        deferred = []

        def evac_block(pr, q0, augA, augB):
            # critical part: free the aug banks fast (sums + unnormalized
            # evacuation); reciprocal + normalization are deferred off the
            # critical path.
            srow_t = srp.tile([1, 1024], F32, tag="sr", name="srow_t")
            nc.vector.tensor_copy(out=srow_t[:, 0:512], in_=augA[HD:HD + 1, :])
            nc.vector.tensor_copy(out=srow_t[:, 512:1024], in_=augB[HD:HD + 1, :])
            nc.vector.reciprocal_approx_fast(out=srow_t, in_=srow_t)
            dstA = pairs_sb[0:64, pr, q0:q0 + 512]
            dstB = pairs_sb[64:128, pr, q0:q0 + 512]
            nc.vector.tensor_copy(out=dstA, in_=augA[:HD, :])
            nc.vector.tensor_copy(out=dstB, in_=augB[:HD, :])

            def norm():
                # full-height broadcast: each half of the normalize multiply
                # needs in1 at the same base partition as its data (walrus
                # requires equal SBUF base partitions for TensorTensor).
                rep_t = repp.tile([P, 1024], F32, tag="rep", name="rep_t")
                nc.gpsimd.partition_broadcast(rep_t, srow_t)
                nc.vector.tensor_tensor(dstA, dstA, rep_t[0:64, 0:512], ALU.mult)
                nc.vector.tensor_tensor(dstB, dstB, rep_t[64:128, 512:1024],
                                        ALU.mult)
            deferred.append(norm)

        # hook schedule over global slots (pair pr covers [pr*NB*NKK, ...))
        SLOTS = NB * NKK
        hooks = {}

        def add_hook(slot, fn):
            hooks.setdefault(slot, []).append(fn)

        for kk in range(NKK):
            add_hook(kk, (lambda m2: lambda: v_group(m2))(kk))
        for n in range(1, NB):           # k(0, n) needed from slot n*4
            add_hook(max(0, 4 * n - 3), (lambda a: lambda: qk_group(0, 1, a))(n))
        for qb in range(1, NB):          # q(0, qb) needed at slot qb*NKK
            add_hook(qb * NKK - 8, (lambda a: lambda: qk_group(0, 0, a))(qb))
        for m in range(1, NM):           # qk(pair m) during pair m-1
            base = (m - 1) * SLOTS + (28 if m == 1 else 4)
            for g, (mat, n) in enumerate((mt, nn) for mt in (0, 1)
                                         for nn in range(NB)):
                add_hook(min(base + 3 * g, m * SLOTS - 1),
                         (lambda a, b, c: lambda: qk_group(a, b, c))(m, mat, n))
        if S // P >= 8:                  # fc for block qb-1 during pair-3 qb
            for qb in range(1, NB):
                for i, (qi, ot) in enumerate((qi, ot) for qi in range(4)
                                             for ot in range(D // 512)):
                    add_hook((NM - 1) * SLOTS + qb * NKK + 2 * i + 3,
                             (lambda a, b: lambda: fc_group(a, b))
                             ((qb - 1) * 4 + qi, ot))

        steps = [(pr, qb, kk) for pr in range(NM) for qb in range(NB)
                 for kk in range(NKK)]
        prev = None        # (p_t, kk, augA, augB, pr, q0)
        augA = augB = None
        for si, (pr, qb, kk) in enumerate(steps):
            if deferred and kk >= 2:
                deferred.pop(0)()
            pending = hooks.pop(si, ())
            qT_t, kT_t = qk_pair_tiles(pr)
            q0 = qb * 512
            if kk == 0:
                augA = paug.tile([P, 512], F32, tag="augA", name="augA")
                augB = paug.tile([P, 512], F32, tag="augB", name="augB")
            ps_t = psp.tile([P, 1024], F32, tag="ps", name="ps_s")
            nc.tensor.matmul(ps_t[:, 0:512],
                             kT_t[0:64, kk * P:(kk + 1) * P],
                             qT_t[0:64, q0:q0 + 512], start=True, stop=True)
            nc.tensor.matmul(ps_t[:, 512:1024],
                             kT_t[64:128, kk * P:(kk + 1) * P],
                             qT_t[64:128, q0:q0 + 512], start=True, stop=True)
            if prev is not None:
                p_p, kkp, aA, aB, prp, q0p = prev
                nc.tensor.matmul(aA[:HD + 1, :], v_sb[:, kkp, 2 * prp, :],
                                 p_p[:, 0:512],
                                 start=(kkp == 0), stop=(kkp == NKK - 1))
                nc.tensor.matmul(aB[:HD + 1, :], v_sb[:, kkp, 2 * prp + 1, :],
                                 p_p[:, 512:1024],
                                 start=(kkp == 0), stop=(kkp == NKK - 1))
            p_t = work.tile([P, 1024], F16, tag="p", name="p_t", bufs=3)
            nc.scalar.activation(p_t, ps_t, AFT.Exp, scale=0.125)
            m_ap = mT_sb[:, kk, q0:q0 + 512]
            nc.vector.tensor_tensor(p_t[:, 0:512], p_t[:, 0:512], m_ap, ALU.mult)
            nc.vector.tensor_tensor(p_t[:, 512:1024], p_t[:, 512:1024], m_ap,
                                    ALU.mult)
            if prev is not None and prev[1] == NKK - 1:
                evac_block(prev[4], prev[5], prev[2], prev[3])
            for fn in pending:
                fn()
            prev = (p_t, kk, augA, augB, pr, q0)
        p_p, kkp, aA, aB, prp, q0p = prev
        nc.tensor.matmul(aA[:HD + 1, :], v_sb[:, kkp, 2 * prp, :],
                         p_p[:, 0:512], start=(kkp == 0), stop=True)
        nc.tensor.matmul(aB[:HD + 1, :], v_sb[:, kkp, 2 * prp + 1, :],
                         p_p[:, 512:1024], start=(kkp == 0), stop=True)
        evac_block(prp, q0p, aA, aB)
        while deferred:
            deferred.pop(0)()
        for fns in [hooks[k] for k in sorted(hooks)]:
            for fn in fns:
                fn()

        # fc groups flushed here cover the last qq block (the rest ran
        # as hooks inside pair 3's attention loop)
        for qt in range((S // P) - 4 if S // P >= 8 else 0, S // P):
            for ot in range(D // 512):
                fc_group(qt, ot)


_prog_cache = {}


def _get_program(S=S_FULL):
    if S not in _prog_cache:
        _prog_cache[S] = build_program(S)
    return _prog_cache[S]


def make_in_maps(x, mask, Wq, bq, Wk, bk, Wv, bv, Wf, bf):
    """Host-side sharding + layout prep (fp16 casts / transposes)."""
    in_maps = []
    WqT = np.ascontiguousarray(np.asarray(Wq).T.astype(np.float16))
    WkT = np.ascontiguousarray(np.asarray(Wk).T.astype(np.float16))
    WvT = np.ascontiguousarray(np.asarray(Wv).T.astype(np.float16))
    WfT = np.asarray(Wf).T.astype(np.float16)
    bq = np.asarray(bq, dtype=np.float32)
    bk = np.asarray(bk, dtype=np.float32)
    bv = np.asarray(bv, dtype=np.float32)
    x = np.asarray(x)
    mask = np.asarray(mask)
    xT_cache, mT_cache = {}, {}
    for c in range(NCORES):
        b, hb = c // 2, c % 2
        if b not in xT_cache:
            xT_cache[b] = np.ascontiguousarray(x[b].T.astype(np.float16))
            mT_cache[b] = np.ascontiguousarray(mask[b, 0].T.astype(np.float16))
        hs = slice(hb * DPC, (hb + 1) * DPC)
        in_maps.append({
            "xT": xT_cache[b],
            "maskT": mT_cache[b],
            "wqT": np.ascontiguousarray(WqT[:, hs]),
            "wkT": np.ascontiguousarray(WkT[:, hs]),
            "wvT": np.ascontiguousarray(WvT[:, hs]),
            "wfT": np.ascontiguousarray(WfT[hs, :]),
            "bq": np.ascontiguousarray(bq[hs]),
            "bk": np.ascontiguousarray(bk[hs]),
            "bvr": np.ascontiguousarray(
                np.tile(bv[hs][None, :].astype(np.float16), (P, 1))),
        })
    return in_maps


def kernel(x, mask, Wq, bq, Wk, bk, Wv, bv, Wf, bf, _trace=False):
    nc = _get_program()
    in_maps = make_in_maps(x, mask, Wq, bq, Wk, bk, Wv, bv, Wf, bf)
    res = run_bass_kernel_spmd(nc, in_maps, core_ids=list(range(NCORES)),
                               trace=_trace)
    out = np.empty((B, S_FULL, D), np.float32)
    bf32 = np.asarray(bf, dtype=np.float32)
    for b in range(B):
        out[b] = res.results[2 * b]["y"] + res.results[2 * b + 1]["y"] + bf32[None, :]
    if _trace:
        kernel._last_results = res
    return out
